# revision 3
# baseline (speedup 1.0000x reference)
"""MoE QLoRA linear kernel for Trainium2 (8 NeuronCores, data-parallel over tokens).

Computes, for x:(B,S,IN) f32:
    base  = x @ W.T + b
    gates = softmax(x @ Wr.T)                       # (tok, E)
    proj  = x @ A[e].T                              # (tok, E, R)
    out   = base + sum_e SCALE * gates[...,e] * (proj[...,e,:] @ Bm[e].T)

Key algebraic fold: the gated expert mix is a single rank-(E*R) matmul:
    wproj[t, er] = SCALE * gates[t, e] * proj[t, er]          (er = e*R+r)
    lora[t, o]   = sum_er wproj[t, er] * Bcat[er, o]          (Bcat[er,o] = Bm[e,o,r])
and the bias b is folded in as an extra contraction row (wproj row of ones,
Bcat row = b), so base+lora+bias all accumulate in one PSUM group on the PE.

Per-core kernel (1024 tokens), everything oriented (feature-partition, token-free):
  phase 1: PSUM(36,512) = [A;Wr]^T-stationary matmuls over 32 k-tiles ->
           proj rows 0..31, router logits rows 32..35; softmax via exp +
           PE ones-matmul partition reductions/broadcasts; wproj written fp16.
           The o-tile-0 base k-loop is emitted between the proj matmuls and
           the gating chain so the PE stays busy while ACT/DVE run softmax.
  phase 2: for each of 32 o-tiles: out(128o, t) = W-tile-stationary matmul
           over 32 k-tiles + one lora matmul (k=33) accumulated into PSUM,
           copy to SBUF, DMA out as (OUT, tok); host transposes back.

All matmul inputs are fp16 (host-cast; PE runs fp16 at full bf16 rate,
fp32 PSUM accumulation). Host pre-tiles all layouts so every DMA is
contiguous and the kernel needs zero on-chip transposes.

Perf note: this toolchain's walrus runs with --enable-ldw-opt=false (=true
crashes codegen), so every matmul gets its own LDWEIGHTS (~53ns each,
~115us/core) on top of the ~490us cost-model prediction; measured device
time is ~600-650us/core. Matmul count (2182) is at the hardware floor
(m<=128, n<=512/PSUM bank, k<=128), so no tiling change reduces it. The
timeline trace shows the PE sequencer saturated end-to-end; PE-engine idle
is only startup DMA (~11us, both alternate DMA rings measured worse) and
the framework tail drain (~5us).

Known unexploited optimization (identified, not landed): the post-Tile BIR
contains 1068 redundant consecutive InstLdweights (second load of each
same-stationary (o-tile, k) pair), all wait/update-free and hence deletable
for ~57us (~9%). Landing it requires deleting them from the compiled
module's PE instruction stream (walrus pairs a standalone InstLdweights
with following non-self-loading matmuls for 2-byte dtypes), then full
hardware re-validation of numerics on all 8 cores.
"""

import numpy as np

import concourse.bass as bass
import concourse.tile as tile
from concourse import bacc, mybir
from concourse import bass_utils

# Problem shape (hardcoded; kernel.py must be self-contained)
B, S, IN, OUT, E, R = 4, 2048, 4096, 4096, 4, 8
SCALE = 16.0 / 8.0
N_CORES = 8
TOK = B * S                  # 8192 tokens
TPC = TOK // N_CORES         # 1024 tokens per core
P = 128                      # partitions
KT = IN // P                 # 32 k-tiles (contraction)
OT = OUT // P                # 32 output tiles
NSLAB = 512                  # moving-operand free size (PSUM bank = 512 f32)
NS = TPC // NSLAB            # 2 token slabs per core
ER = E * R                   # 32 low-rank rows
ERA = ER + 1                 # +1 ones row (bias fold)

F16 = mybir.dt.float16
F32 = mybir.dt.float32

_NC = None


def build_nc(reps=1, ns=NS):
    NS_ = ns
    nc = bacc.Bacc("TRN2", target_bir_lowering=False, debug=False)

    xd = nc.dram_tensor("xd", [P, KT, TPC], F16, kind="ExternalInput")
    wd = nc.dram_tensor("wd", [OT, P, KT, P], F16, kind="ExternalInput")
    artd = nc.dram_tensor("artd", [P, KT, ER + E], F16, kind="ExternalInput")
    btd = nc.dram_tensor("btd", [ERA, OUT], F16, kind="ExternalInput")
    seld = nc.dram_tensor("seld", [E, ER], F32, kind="ExternalInput")
    od = nc.dram_tensor("od", [OUT, TPC], F32, kind="ExternalOutput")

    with tile.TileContext(nc) as tc:
        with (
            tc.tile_pool(name="consts", bufs=1) as consts,
            tc.tile_pool(name="wpool", bufs=3) as wpool,
            tc.tile_pool(name="opool", bufs=3) as opool,
            tc.tile_pool(name="small", bufs=2) as small,
            tc.tile_pool(name="psum_proj", bufs=1, space="PSUM") as psum_proj,
            tc.tile_pool(name="psum_base", bufs=2, space="PSUM") as psum_base,
        ):
            art_sb = consts.tile([P, KT, ER + E], F16)
            nc.sync.dma_start(out=art_sb[:], in_=artd[:])
            bt_sb = consts.tile([ERA, OUT], F16)
            nc.sync.dma_start(out=bt_sb[:], in_=btd[:])
            sel_sb = consts.tile([E, ER], F32)
            nc.sync.dma_start(out=sel_sb[:], in_=seld[:])

            w_tiles = {}

            def load_w(ot):
                w_sb = wpool.tile([P, KT, P], F16, tag="w", name="w_sb")
                nc.sync.dma_start(out=w_sb[:], in_=wd[ot])
                w_tiles[ot] = w_sb

            # first two W tiles before the bulk x load: o-tile 0 can start
            # as soon as phase-1 finishes on the PE
            load_w(0)
            load_w(1)

            # Resident activations: x^T tiled (p=i%128, k=i//128, t), fp16, 8 MiB.
            x_sb = consts.tile([P, KT, TPC], F16)
            for k in range(KT):
                nc.sync.dma_start(out=x_sb[:, k, :], in_=xd[:, k, :])

            ones_e1 = consts.tile([E, 1], F32)
            nc.vector.memset(ones_e1[:], 1.0)
            ones_1e = consts.tile([1, E], F32)
            nc.vector.memset(ones_1e[:], 1.0)
            # Gated low-rank projection, fp16, rows 0..31 = wproj, row 32 = ones.
            wp_sb = consts.tile([ERA, TPC], F16)
            nc.vector.memset(wp_sb[ER : ER + 1, :], 1.0)

            # ---------- phase 1: proj + router matmuls ----------
            pps = []
            for t in range(NS_):
                tsl = slice(t * NSLAB, (t + 1) * NSLAB)
                # rows 0..31: proj^T (er, t); rows 32..35: router logits (e, t)
                pp = psum_proj.tile(
                    [ER + E, NSLAB], F32, tag=f"pp{t}", name=f"pp{t}"
                )
                for k in range(KT):
                    nc.tensor.matmul(
                        pp[:],
                        art_sb[:, k, :],
                        x_sb[:, k, tsl],
                        start=(k == 0),
                        stop=(k == KT - 1),
                    )
                pps.append(pp)

            def gating(t):
                # softmax over the 4 expert rows (no max-sub: |logit| < ~8),
                # partition reductions/broadcasts done with tiny PE matmuls
                tsl = slice(t * NSLAB, (t + 1) * NSLAB)
                pp = pps[t]
                e_sb = small.tile([E, NSLAB], F32, tag="e", name="e_sb")
                nc.scalar.activation(
                    e_sb[:], pp[ER : ER + E, :], mybir.ActivationFunctionType.Exp
                )
                s_ps = psum_proj.tile([1, NSLAB], F32, tag="gat", name="s_ps")
                nc.tensor.matmul(s_ps[:], ones_e1[:], e_sb[:])  # sum_e exp
                r_sb = small.tile([1, NSLAB], F32, tag="r", name="r_sb")
                nc.vector.reciprocal(r_sb[:], s_ps[:])
                r4_ps = psum_proj.tile([E, NSLAB], F32, tag="gat", name="r4_ps")
                nc.tensor.matmul(r4_ps[:], ones_1e[:], r_sb[:])  # bcast to 4 rows
                g4_sb = small.tile([E, NSLAB], F32, tag="g4", name="g4_sb")
                nc.vector.tensor_mul(g4_sb[:], e_sb[:], r4_ps[:])
                # (SCALE * gate)[er, t] via 0/1*SCALE selection matmul
                g32_ps = psum_proj.tile([ER, NSLAB], F32, tag="gat", name="g32_ps")
                nc.tensor.matmul(g32_ps[:], sel_sb[:], g4_sb[:])
                # walrus: tensor_tensor may read at most one operand from PSUM
                g32_sb = small.tile([ER, NSLAB], F32, tag="g32s", name="g32_sb")
                nc.vector.tensor_copy(g32_sb[:], g32_ps[:])
                nc.vector.tensor_mul(wp_sb[0:ER, tsl], pp[0:ER, :], g32_sb[:])

            # ---------- phase 2: base matmul + lora + bias ----------
            def base_kloop(ot):
                if ot not in w_tiles:
                    load_w(ot)
                pots = [
                    psum_base.tile([P, NSLAB], F32, tag=f"po{t}", name=f"po{t}")
                    for t in range(NS_)
                ]
                for k in range(KT):
                    for t in range(NS_):
                        nc.tensor.matmul(
                            pots[t][:],
                            w_tiles[ot][:, k, :],
                            x_sb[:, k, t * NSLAB : (t + 1) * NSLAB],
                            start=(k == 0),
                            stop=False,
                        )
                return pots

            def base_tail(ot, pots):
                osl = slice(ot * P, (ot + 1) * P)
                for t in range(NS_):
                    nc.tensor.matmul(
                        pots[t][:],
                        bt_sb[:, osl],
                        wp_sb[:, t * NSLAB : (t + 1) * NSLAB],
                        start=False,
                        stop=True,
                    )
                o_sb = opool.tile([P, TPC], F32, tag="o", name="o_sb")
                for t in range(NS_):
                    nc.vector.tensor_copy(
                        o_sb[:, t * NSLAB : (t + 1) * NSLAB], pots[t][:]
                    )
                nc.sync.dma_start(out=od[osl, :], in_=o_sb[:])
                del w_tiles[ot]

            for rep in range(reps):
                if rep == 0:
                    # o-tile 0's k-loop keeps the PE busy during the gating chain
                    pots0 = base_kloop(0)
                    for t in range(NS_):
                        gating(t)
                    base_tail(0, pots0)
                    start_ot = 1
                else:
                    start_ot = 0
                for ot in range(start_ot, OT):
                    pots = base_kloop(ot)
                    base_tail(ot, pots)

    nc.compile()
    dedup_ldweights(nc)
    return nc


def dedup_ldweights(nc):
    """Delete redundant consecutive InstLdweights from the PE stream.

    After bacc lowering every matmul is a standalone InstLdweights followed
    by a non-self-loading InstMatmult.  When two MMs in a row use the same
    stationary tile (the two 512-token slabs of one (o-tile, k) pair), the
    second load is a no-op reload of weights already in the array.  Deleting
    it is safe when (a) its full AP signature matches the immediately
    preceding InstLdweights, (b) only non-self-loading InstMatmults sit
    between them (nothing else touched the array or SBUF ordering), and
    (c) the load carries no semaphore waits/updates.  Any rewrite of the
    underlying SBUF region is ordered after the *later* consumers by the
    tile framework's region tracking, so the weight contents cannot change
    between the two loads.
    """
    from concourse import mybir

    def sig(i):
        ap = i.ins[0]
        return (
            ap.memref,
            ap.offset,
            str(ap.ap),
            str(ap.dtype),
            str(i.tile_position),
            str(i.tile_size),
            str(i.perf_mode),
            str(i.is_transpose),
        )

    removed = 0
    for b in nc.m.functions[0].blocks:
        last = None
        keep = []
        for i in b.instructions:
            if i.engine != mybir.EngineType.PE:
                keep.append(i)
                continue
            if isinstance(i, mybir.InstLdweights):
                si = i.sync_info
                clean = si is None or (not si.on_wait and not si.on_update)
                if clean and last is not None and sig(i) == last:
                    removed += 1
                    continue  # drop the redundant reload
                last = sig(i)
                keep.append(i)
            elif (
                isinstance(i, mybir.InstMatmult)
                and getattr(i, "ldweights", None) is False
            ):
                keep.append(i)
            else:
                last = None
                keep.append(i)
        b.instructions[:] = keep
    return removed


def get_nc():
    global _NC
    if _NC is None:
        _NC = build_nc()
    return _NC


def _prep_shared(W, b, A, Bm, Wr):
    # W (OUT, IN) -> wd[ot, p, k, o] = W[ot*128+o, k*128+p], fp16, contiguous
    wd = np.ascontiguousarray(
        W.reshape(OT, P, KT, P).transpose(0, 3, 2, 1).astype(np.float16)
    )
    # [A (E,R,IN) flattened; Wr (E,IN)] -> art[p, k, j] = AR[j, k*128+p]
    ar = np.concatenate([A.reshape(ER, IN), Wr], axis=0)  # (36, IN)
    artd = np.ascontiguousarray(
        ar.T.reshape(KT, P, ER + E).transpose(1, 0, 2).astype(np.float16)
    )
    # Bcat rows er = Bm[e,:,r]; row 32 = bias
    bt = np.concatenate([Bm.transpose(0, 2, 1).reshape(ER, OUT), b[None, :]], axis=0)
    btd = np.ascontiguousarray(bt.astype(np.float16))
    sel = np.zeros((E, ER), np.float32)
    for e in range(E):
        sel[e, e * R : (e + 1) * R] = SCALE
    return wd, artd, btd, sel


def _prep_x_shard(xt, c):
    xs = xt[c * TPC : (c + 1) * TPC]  # (TPC, IN)
    return np.ascontiguousarray(
        xs.T.reshape(KT, P, TPC).transpose(1, 0, 2).astype(np.float16)
    )


def make_in_maps(x, W, b, A, Bm, Wr):
    xt = np.asarray(x, np.float32).reshape(TOK, IN)
    wd, artd, btd, sel = _prep_shared(
        np.asarray(W, np.float32),
        np.asarray(b, np.float32),
        np.asarray(A, np.float32),
        np.asarray(Bm, np.float32),
        np.asarray(Wr, np.float32),
    )
    return [
        {
            "xd": _prep_x_shard(xt, c),
            "wd": wd,
            "artd": artd,
            "btd": btd,
            "seld": sel,
        }
        for c in range(N_CORES)
    ]


def gather_out(results):
    # per-core od is (OUT, TPC); tokens are sharded contiguously
    return np.concatenate([r["od"].T for r in results], axis=0).reshape(B, S, OUT)


def kernel(x, W, b, A, Bm, Wr, _trace=False):
    nc = get_nc()
    in_maps = make_in_maps(x, W, b, A, Bm, Wr)
    res = bass_utils.run_bass_kernel_spmd(
        nc, in_maps, core_ids=list(range(N_CORES)), trace=_trace
    )
    out = gather_out(res.results)
    if _trace:
        return out, res
    return out



# revision 9
# speedup vs baseline: 1.0294x; 1.0294x over previous
"""MoE QLoRA linear kernel for Trainium2 (8 NeuronCores, data-parallel over tokens).

Computes, for x:(B,S,IN) f32:
    base  = x @ W.T + b
    gates = softmax(x @ Wr.T)                       # (tok, E)
    proj  = x @ A[e].T                              # (tok, E, R)
    out   = base + sum_e SCALE * gates[...,e] * (proj[...,e,:] @ Bm[e].T)

Key algebraic fold: the gated expert mix is a single rank-(E*R) matmul:
    wproj[t, er] = SCALE * gates[t, e] * proj[t, er]          (er = e*R+r)
    lora[t, o]   = sum_er wproj[t, er] * Bcat[er, o]          (Bcat[er,o] = Bm[e,o,r])
and the bias b is folded in as an extra contraction row (wproj row of ones,
Bcat row = b), so base+lora+bias all accumulate in one PSUM group on the PE.

Per-core kernel (1024 tokens), everything oriented (feature-partition, token-free):
  phase 1: PSUM(36,512) = [A;Wr]^T-stationary matmuls over 32 k-tiles ->
           proj rows 0..31, router logits rows 32..35; softmax via exp +
           PE ones-matmul partition reductions/broadcasts; wproj written fp16.
           The o-tile-0 base k-loop is emitted between the proj matmuls and
           the gating chain so the PE stays busy while ACT/DVE run softmax.
  phase 2: for each of 32 o-tiles: out(128o, t) = W-tile-stationary matmul
           over 32 k-tiles + one lora matmul (k=33) accumulated into PSUM,
           copy to SBUF, DMA out as (OUT, tok); host transposes back.

All matmul inputs are fp16 (host-cast; PE runs fp16 at full bf16 rate,
fp32 PSUM accumulation). Host pre-tiles all layouts so every DMA is
contiguous and the kernel needs zero on-chip transposes.

Perf note: this toolchain's walrus runs with --enable-ldw-opt=false (=true
crashes codegen), so every matmul gets its own LDWEIGHTS (~53ns each,
~115us/core) on top of the ~490us cost-model prediction; measured device
time is ~600-650us/core. Matmul count (2182) is at the hardware floor
(m<=128, n<=512/PSUM bank, k<=128), so no tiling change reduces it. The
timeline trace shows the PE sequencer saturated end-to-end; PE-engine idle
is only startup DMA (~11us, both alternate DMA rings measured worse) and
the framework tail drain (~5us).

Known unexploited optimization (identified, not landed): the post-Tile BIR
contains 1068 redundant consecutive InstLdweights (second load of each
same-stationary (o-tile, k) pair), all wait/update-free and hence deletable
for ~57us (~9%). Landing it requires deleting them from the compiled
module's PE instruction stream (walrus pairs a standalone InstLdweights
with following non-self-loading matmuls for 2-byte dtypes), then full
hardware re-validation of numerics on all 8 cores.
"""

import numpy as np

import concourse.bass as bass
import concourse.tile as tile
from concourse import bacc, mybir
from concourse import bass_utils

# Problem shape (hardcoded; kernel.py must be self-contained)
B, S, IN, OUT, E, R = 4, 2048, 4096, 4096, 4, 8
SCALE = 16.0 / 8.0
N_CORES = 8
TOK = B * S                  # 8192 tokens
TPC = TOK // N_CORES         # 1024 tokens per core
P = 128                      # partitions
KT = IN // P                 # 32 k-tiles (contraction)
OT = OUT // P                # 32 output tiles
NSLAB = 512                  # moving-operand free size (PSUM bank = 512 f32)
NS = TPC // NSLAB            # 2 token slabs per core
ER = E * R                   # 32 low-rank rows
ERA = ER + 1                 # +1 ones row (bias fold)

F16 = mybir.dt.float16
F32 = mybir.dt.float32

_NC = None

# Optional post-compile transform hook (used by experiments; None in prod).
POST_COMPILE = None


def build_nc(reps=1, ns=NS):
    NS_ = ns
    nc = bacc.Bacc("TRN2", target_bir_lowering=False, debug=False)

    xd = nc.dram_tensor("xd", [P, KT, TPC], F16, kind="ExternalInput")
    wd = nc.dram_tensor("wd", [OT, P, KT, P], F16, kind="ExternalInput")
    artd = nc.dram_tensor("artd", [P, KT, ER + E], F16, kind="ExternalInput")
    btd = nc.dram_tensor("btd", [ERA, OUT], F16, kind="ExternalInput")
    seld = nc.dram_tensor("seld", [E, ER], F32, kind="ExternalInput")
    od = nc.dram_tensor("od", [OUT, TPC], F32, kind="ExternalOutput")

    with tile.TileContext(nc) as tc:
        with (
            tc.tile_pool(name="consts", bufs=1) as consts,
            tc.tile_pool(name="wpool", bufs=3) as wpool,
            tc.tile_pool(name="opool", bufs=3) as opool,
            tc.tile_pool(name="small", bufs=2) as small,
            tc.tile_pool(name="psum_proj", bufs=1, space="PSUM") as psum_proj,
            tc.tile_pool(name="psum_base", bufs=2, space="PSUM") as psum_base,
        ):
            art_sb = consts.tile([P, KT, ER + E], F16)
            nc.sync.dma_start(out=art_sb[:], in_=artd[:])
            bt_sb = consts.tile([ERA, OUT], F16)
            nc.sync.dma_start(out=bt_sb[:], in_=btd[:])
            sel_sb = consts.tile([E, ER], F32)
            nc.sync.dma_start(out=sel_sb[:], in_=seld[:])

            w_tiles = {}

            def load_w(ot):
                w_sb = wpool.tile([P, KT, P], F16, tag="w", name="w_sb")
                nc.sync.dma_start(out=w_sb[:], in_=wd[ot])
                w_tiles[ot] = w_sb

            # first two W tiles before the bulk x load: o-tile 0 can start
            # as soon as phase-1 finishes on the PE
            load_w(0)
            load_w(1)

            # Resident activations: x^T tiled (p=i%128, k=i//128, t), fp16, 8 MiB.
            x_sb = consts.tile([P, KT, TPC], F16)
            for k in range(KT):
                nc.sync.dma_start(out=x_sb[:, k, :], in_=xd[:, k, :])

            ones_e1 = consts.tile([E, 1], F32)
            nc.vector.memset(ones_e1[:], 1.0)
            ones_1e = consts.tile([1, E], F32)
            nc.vector.memset(ones_1e[:], 1.0)
            # Gated low-rank projection, fp16, rows 0..31 = wproj, row 32 = ones.
            wp_sb = consts.tile([ERA, TPC], F16)
            nc.vector.memset(wp_sb[ER : ER + 1, :], 1.0)

            # ---------- phase 1: proj + router matmuls ----------
            pps = []
            for t in range(NS_):
                tsl = slice(t * NSLAB, (t + 1) * NSLAB)
                # rows 0..31: proj^T (er, t); rows 32..35: router logits (e, t)
                pp = psum_proj.tile(
                    [ER + E, NSLAB], F32, tag=f"pp{t}", name=f"pp{t}"
                )
                for k in range(KT):
                    nc.tensor.matmul(
                        pp[:],
                        art_sb[:, k, :],
                        x_sb[:, k, tsl],
                        start=(k == 0),
                        stop=(k == KT - 1),
                    )
                pps.append(pp)

            def gating(t):
                # softmax over the 4 expert rows (no max-sub: |logit| < ~8),
                # partition reductions/broadcasts done with tiny PE matmuls
                tsl = slice(t * NSLAB, (t + 1) * NSLAB)
                pp = pps[t]
                e_sb = small.tile([E, NSLAB], F32, tag="e", name="e_sb")
                nc.scalar.activation(
                    e_sb[:], pp[ER : ER + E, :], mybir.ActivationFunctionType.Exp
                )
                s_ps = psum_proj.tile([1, NSLAB], F32, tag="gat", name="s_ps")
                nc.tensor.matmul(s_ps[:], ones_e1[:], e_sb[:])  # sum_e exp
                r_sb = small.tile([1, NSLAB], F32, tag="r", name="r_sb")
                nc.vector.reciprocal(r_sb[:], s_ps[:])
                r4_ps = psum_proj.tile([E, NSLAB], F32, tag="gat", name="r4_ps")
                nc.tensor.matmul(r4_ps[:], ones_1e[:], r_sb[:])  # bcast to 4 rows
                g4_sb = small.tile([E, NSLAB], F32, tag="g4", name="g4_sb")
                nc.vector.tensor_mul(g4_sb[:], e_sb[:], r4_ps[:])
                # (SCALE * gate)[er, t] via 0/1*SCALE selection matmul
                g32_ps = psum_proj.tile([ER, NSLAB], F32, tag="gat", name="g32_ps")
                nc.tensor.matmul(g32_ps[:], sel_sb[:], g4_sb[:])
                # walrus: tensor_tensor may read at most one operand from PSUM
                g32_sb = small.tile([ER, NSLAB], F32, tag="g32s", name="g32_sb")
                nc.vector.tensor_copy(g32_sb[:], g32_ps[:])
                nc.vector.tensor_mul(wp_sb[0:ER, tsl], pp[0:ER, :], g32_sb[:])

            # ---------- phase 2: base matmul + lora + bias ----------
            def base_kloop(ot):
                if ot not in w_tiles:
                    load_w(ot)
                pots = [
                    psum_base.tile([P, NSLAB], F32, tag=f"po{t}", name=f"po{t}")
                    for t in range(NS_)
                ]
                for k in range(KT):
                    for t in range(NS_):
                        nc.tensor.matmul(
                            pots[t][:],
                            w_tiles[ot][:, k, :],
                            x_sb[:, k, t * NSLAB : (t + 1) * NSLAB],
                            start=(k == 0),
                            stop=False,
                        )
                return pots

            def base_tail(ot, pots):
                osl = slice(ot * P, (ot + 1) * P)
                for t in range(NS_):
                    nc.tensor.matmul(
                        pots[t][:],
                        bt_sb[:, osl],
                        wp_sb[:, t * NSLAB : (t + 1) * NSLAB],
                        start=False,
                        stop=True,
                    )
                o_sb = opool.tile([P, TPC], F32, tag="o", name="o_sb")
                for t in range(NS_):
                    nc.vector.tensor_copy(
                        o_sb[:, t * NSLAB : (t + 1) * NSLAB], pots[t][:]
                    )
                nc.sync.dma_start(out=od[osl, :], in_=o_sb[:])
                del w_tiles[ot]

            for rep in range(reps):
                if rep == 0:
                    # o-tile 0's k-loop keeps the PE busy during the gating chain
                    pots0 = base_kloop(0)
                    for t in range(NS_):
                        gating(t)
                    base_tail(0, pots0)
                    start_ot = 1
                else:
                    start_ot = 0
                for ot in range(start_ot, OT):
                    pots = base_kloop(ot)
                    base_tail(ot, pots)

    nc.compile()
    if POST_COMPILE is not None:
        POST_COMPILE(nc)
    else:
        prune_mm_updates(nc)
    return nc


def dedup_ldweights(nc):
    """Delete redundant consecutive InstLdweights from the PE stream.

    After bacc lowering every matmul is a standalone InstLdweights followed
    by a non-self-loading InstMatmult.  When two MMs in a row use the same
    stationary tile (the two 512-token slabs of one (o-tile, k) pair), the
    second load is a no-op reload of weights already in the array.  Deleting
    it is safe when (a) its full AP signature matches the immediately
    preceding InstLdweights, (b) only non-self-loading InstMatmults sit
    between them (nothing else touched the array or SBUF ordering), and
    (c) the load carries no semaphore waits/updates.  Any rewrite of the
    underlying SBUF region is ordered after the *later* consumers by the
    tile framework's region tracking, so the weight contents cannot change
    between the two loads.
    """
    from concourse import mybir

    def sig(i):
        ap = i.ins[0]
        return (
            ap.memref,
            ap.offset,
            str(ap.ap),
            str(ap.dtype),
            str(i.tile_position),
            str(i.tile_size),
            str(i.perf_mode),
            str(i.is_transpose),
        )

    removed = 0
    for b in nc.m.functions[0].blocks:
        last = None
        keep = []
        for i in b.instructions:
            if i.engine != mybir.EngineType.PE:
                keep.append(i)
                continue
            if isinstance(i, mybir.InstLdweights):
                si = i.sync_info
                clean = si is None or (not si.on_wait and not si.on_update)
                if clean and last is not None and sig(i) == last:
                    removed += 1
                    continue  # drop the redundant reload
                last = sig(i)
                keep.append(i)
            elif (
                isinstance(i, mybir.InstMatmult)
                and getattr(i, "ldweights", None) is False
            ):
                keep.append(i)
            else:
                last = None
                keep.append(i)
        b.instructions[:] = keep
    return removed


def prune_mm_updates(nc):
    """Remove PE-matmul sem updates that no waiter's threshold references.

    Every tile-emitted matmul carries a `PE_sem++@complete`; the hardware
    retires that sem write on the PE sequencer, and when the next instruction
    is another matmul the write stalls its issue (~40ns measured: bare MM->MM
    gap 91ns vs 53ns with an LDWEIGHTS between).  Waiters (DVE copies, DMA
    recycles) reference only ~100 distinct cumulative counts, so all other
    increments are unobservable.  Keep an update only at referenced
    positions and renumber every wait threshold to the new cumulative count.
    Kept updates remain on the same instructions, so every waiter still
    unblocks at exactly the same matmul completion as before.
    """
    from concourse import mybir

    SEM_CANDIDATES = []
    # find sems updated exclusively by PE InstMatmult via sem-inc
    upd_by = {}
    for f in nc.m.functions:
        for b in f.blocks:
            for i in b.instructions:
                si = i.sync_info
                if not si:
                    continue
                for u in si.on_update:
                    rec = upd_by.setdefault(u.id, {"mm": 0, "other": 0})
                    if (
                        i.engine == mybir.EngineType.PE
                        and isinstance(i, mybir.InstMatmult)
                        and u.update_mode == "sem-inc"
                        and (u.update_value or 1) == 1
                    ):
                        rec["mm"] += 1
                    else:
                        rec["other"] += 1
    for sem_id, rec in upd_by.items():
        if rec["mm"] > 0 and rec["other"] == 0:
            SEM_CANDIDATES.append(sem_id)

    total_removed = 0
    for sem_id in SEM_CANDIDATES:
        # ordered updater positions (PE program order across blocks)
        updaters = []  # instruction refs in order
        waits = []  # (SyncWait refs)
        ok = True
        for f in nc.m.functions:
            for b in f.blocks:
                for i in b.instructions:
                    si = i.sync_info
                    if not si:
                        continue
                    for u in si.on_update:
                        if u.id == sem_id:
                            updaters.append(i)
                    for w in si.on_wait:
                        if w.id == sem_id:
                            if (
                                w.wait_mode != "sem-ge-imm"
                                or w.wait_reg is not None
                                or w.wait_value is None
                            ):
                                ok = False
                            waits.append(w)
        if not ok or not waits:
            continue
        n = len(updaters)
        keep = set()
        for w in waits:
            t = w.wait_value
            if t < 1 or t > n:
                ok = False
                break
            keep.add(t)
        if not ok:
            continue
        keep.add(n)  # preserve the final count for any implicit consumers
        # renumber
        sorted_keep = sorted(keep)
        new_count = {}
        for rank, pos in enumerate(sorted_keep, start=1):
            new_count[pos] = rank
        for w in waits:
            w.wait_value = new_count[w.wait_value]
        for pos, inst in enumerate(updaters, start=1):
            if pos not in keep:
                si = inst.sync_info
                si.on_update = [
                    u for u in si.on_update if u.id != sem_id
                ]
                total_removed += 1
    return total_removed


def consolidate_mm_updates(nc):
    """Move per-matmul semaphore completion updates to accumulation-group ends.

    Every matmul the tile framework emits carries a `sem++@complete` update;
    on hardware each update costs ~26ns of PE sequencer time (see the
    pack-tail model in the tensor-engine guide), which over ~2100 matmuls is
    ~55us of pure overhead.  Matmuls complete in program order, so moving the
    increments of the stop=False matmuls of an accumulation group onto the
    group's final stop=True matmul (with a summed update_value) is
    semantically conservative: every waiter still sees the same final counts,
    just potentially a few instructions later.  Waiters whose thresholds fall
    mid-group (e.g. the W-tile DMA recycling a pool buffer) unblock at the
    group end instead — with bufs=3 prefetch headroom that slack is never on
    the critical path.  Only runs of {InstMatmult, InstLdweights} are
    touched; a segment that does not end in a stop=True matmul is left as-is.
    """
    from concourse import mybir

    moved = 0
    for b in nc.m.functions[0].blocks:
        pending = []  # stripped SyncUpdate objects awaiting a stop=True MM
        stripped = []  # (instruction, saved updates) to restore if no flush
        for i in b.instructions:
            if i.engine != mybir.EngineType.PE:
                continue
            if isinstance(i, mybir.InstLdweights):
                continue
            if isinstance(i, mybir.InstMatmult):
                si = i.sync_info
                if i.stop_tensor_calc:
                    if pending:
                        ups = list(si.on_update) if si is not None else []
                        for u in pending:
                            for tgt in ups:
                                if (
                                    tgt.sync_type == u.sync_type
                                    and tgt.id == u.id
                                    and tgt.update_mode == u.update_mode
                                ):
                                    tgt.update_value = (
                                        tgt.update_value or 1
                                    ) + (u.update_value or 1)
                                    break
                            else:
                                ups.append(u)
                        if si is None:
                            i.sync_info = mybir.SyncInfo(
                                on_wait=[], on_update=ups
                            )
                        else:
                            si.on_update = ups
                        pending = []
                        stripped = []
                else:
                    if si is not None and si.on_update:
                        pending.extend(si.on_update)
                        moved += len(si.on_update)
                        stripped.append((i, list(si.on_update)))
                        si.on_update = []
            else:
                # A non-MM PE instruction (event sem, drain, branch) ends the
                # run; restore any updates not yet flushed so ordering
                # relative to this instruction is preserved.
                for inst, ups in stripped:
                    inst.sync_info.on_update = ups
                    moved -= len(ups)
                pending = []
                stripped = []
        for inst, ups in stripped:
            inst.sync_info.on_update = ups
            moved -= len(ups)
    return moved


def get_nc():
    global _NC
    if _NC is None:
        _NC = build_nc()
    return _NC


def _prep_shared(W, b, A, Bm, Wr):
    # W (OUT, IN) -> wd[ot, p, k, o] = W[ot*128+o, k*128+p], fp16, contiguous
    wd = np.ascontiguousarray(
        W.reshape(OT, P, KT, P).transpose(0, 3, 2, 1).astype(np.float16)
    )
    # [A (E,R,IN) flattened; Wr (E,IN)] -> art[p, k, j] = AR[j, k*128+p]
    ar = np.concatenate([A.reshape(ER, IN), Wr], axis=0)  # (36, IN)
    artd = np.ascontiguousarray(
        ar.T.reshape(KT, P, ER + E).transpose(1, 0, 2).astype(np.float16)
    )
    # Bcat rows er = Bm[e,:,r]; row 32 = bias
    bt = np.concatenate([Bm.transpose(0, 2, 1).reshape(ER, OUT), b[None, :]], axis=0)
    btd = np.ascontiguousarray(bt.astype(np.float16))
    sel = np.zeros((E, ER), np.float32)
    for e in range(E):
        sel[e, e * R : (e + 1) * R] = SCALE
    return wd, artd, btd, sel


def _prep_x_shard(xt, c):
    xs = xt[c * TPC : (c + 1) * TPC]  # (TPC, IN)
    return np.ascontiguousarray(
        xs.T.reshape(KT, P, TPC).transpose(1, 0, 2).astype(np.float16)
    )


def make_in_maps(x, W, b, A, Bm, Wr):
    xt = np.asarray(x, np.float32).reshape(TOK, IN)
    wd, artd, btd, sel = _prep_shared(
        np.asarray(W, np.float32),
        np.asarray(b, np.float32),
        np.asarray(A, np.float32),
        np.asarray(Bm, np.float32),
        np.asarray(Wr, np.float32),
    )
    return [
        {
            "xd": _prep_x_shard(xt, c),
            "wd": wd,
            "artd": artd,
            "btd": btd,
            "seld": sel,
        }
        for c in range(N_CORES)
    ]


def gather_out(results):
    # per-core od is (OUT, TPC); tokens are sharded contiguously
    return np.concatenate([r["od"].T for r in results], axis=0).reshape(B, S, OUT)


def kernel(x, W, b, A, Bm, Wr, _trace=False):
    nc = get_nc()
    in_maps = make_in_maps(x, W, b, A, Bm, Wr)
    res = bass_utils.run_bass_kernel_spmd(
        nc, in_maps, core_ids=list(range(N_CORES)), trace=_trace
    )
    out = gather_out(res.results)
    if _trace:
        return out, res
    return out



# revision 10
# speedup vs baseline: 1.0667x; 1.0362x over previous
"""MoE QLoRA linear kernel for Trainium2 (8 NeuronCores, data-parallel over tokens).

Computes, for x:(B,S,IN) f32:
    base  = x @ W.T + b
    gates = softmax(x @ Wr.T)                       # (tok, E)
    proj  = x @ A[e].T                              # (tok, E, R)
    out   = base + sum_e SCALE * gates[...,e] * (proj[...,e,:] @ Bm[e].T)

Key algebraic fold: the gated expert mix is a single rank-(E*R) matmul:
    wproj[t, er] = SCALE * gates[t, e] * proj[t, er]          (er = e*R+r)
    lora[t, o]   = sum_er wproj[t, er] * Bcat[er, o]          (Bcat[er,o] = Bm[e,o,r])
and the bias b is folded in as an extra contraction row (wproj row of ones,
Bcat row = b), so base+lora+bias all accumulate in one PSUM group on the PE.

Per-core kernel (1024 tokens), everything oriented (feature-partition, token-free):
  phase 1: PSUM(36,512) = [A;Wr]^T-stationary matmuls over 32 k-tiles ->
           proj rows 0..31, router logits rows 32..35; softmax via exp +
           PE ones-matmul partition reductions/broadcasts; wproj written fp16.
           The o-tile-0 base k-loop is emitted between the proj matmuls and
           the gating chain so the PE stays busy while ACT/DVE run softmax.
  phase 2: for each of 32 o-tiles: out(128o, t) = W-tile-stationary matmul
           over 32 k-tiles + one lora matmul (k=33) accumulated into PSUM,
           copy to SBUF, DMA out as (OUT, tok); host transposes back.

All matmul inputs are fp16 (host-cast; PE runs fp16 at full bf16 rate,
fp32 PSUM accumulation). Host pre-tiles all layouts so every DMA is
contiguous and the kernel needs zero on-chip transposes.

Perf note: this toolchain's walrus runs with --enable-ldw-opt=false (=true
crashes codegen), so every matmul gets its own LDWEIGHTS (~53ns each,
~115us/core) on top of the ~490us cost-model prediction; measured device
time is ~600-650us/core. Matmul count (2182) is at the hardware floor
(m<=128, n<=512/PSUM bank, k<=128), so no tiling change reduces it. The
timeline trace shows the PE sequencer saturated end-to-end; PE-engine idle
is only startup DMA (~11us, both alternate DMA rings measured worse) and
the framework tail drain (~5us).

Known unexploited optimization (identified, not landed): the post-Tile BIR
contains 1068 redundant consecutive InstLdweights (second load of each
same-stationary (o-tile, k) pair), all wait/update-free and hence deletable
for ~57us (~9%). Landing it requires deleting them from the compiled
module's PE instruction stream (walrus pairs a standalone InstLdweights
with following non-self-loading matmuls for 2-byte dtypes), then full
hardware re-validation of numerics on all 8 cores.
"""

import numpy as np

import concourse.bass as bass
import concourse.tile as tile
from concourse import bacc, mybir
from concourse import bass_utils

# Problem shape (hardcoded; kernel.py must be self-contained)
B, S, IN, OUT, E, R = 4, 2048, 4096, 4096, 4, 8
SCALE = 16.0 / 8.0
N_CORES = 8
TOK = B * S                  # 8192 tokens
TPC = TOK // N_CORES         # 1024 tokens per core
P = 128                      # partitions
KT = IN // P                 # 32 k-tiles (contraction)
OT = OUT // P                # 32 output tiles
NSLAB = 512                  # moving-operand free size (PSUM bank = 512 f32)
NS = TPC // NSLAB            # 2 token slabs per core
ER = E * R                   # 32 low-rank rows
ERA = ER + 1                 # +1 ones row (bias fold)

F16 = mybir.dt.float16
F32 = mybir.dt.float32

_NC = None

# Optional post-compile transform hook (used by experiments; None in prod).
POST_COMPILE = None


def build_nc(reps=1, ns=NS):
    NS_ = ns
    nc = bacc.Bacc("TRN2", target_bir_lowering=False, debug=False)

    xd = nc.dram_tensor("xd", [P, KT, TPC], F16, kind="ExternalInput")
    wd = nc.dram_tensor("wd", [OT, P, KT, P], F16, kind="ExternalInput")
    artd = nc.dram_tensor("artd", [P, KT, ER + E], F16, kind="ExternalInput")
    btd = nc.dram_tensor("btd", [ERA, OUT], F16, kind="ExternalInput")
    seld = nc.dram_tensor("seld", [E, ER], F32, kind="ExternalInput")
    od = nc.dram_tensor("od", [OUT, TPC], F32, kind="ExternalOutput")

    with tile.TileContext(nc) as tc:
        with (
            tc.tile_pool(name="consts", bufs=1) as consts,
            tc.tile_pool(name="wpool", bufs=3) as wpool,
            tc.tile_pool(name="opool", bufs=3) as opool,
            tc.tile_pool(name="small", bufs=2) as small,
            tc.tile_pool(name="psum_proj", bufs=1, space="PSUM") as psum_proj,
            tc.tile_pool(name="psum_base", bufs=2, space="PSUM") as psum_base,
        ):
            art_sb = consts.tile([P, KT, ER + E], F16)
            nc.sync.dma_start(out=art_sb[:], in_=artd[:])
            bt_sb = consts.tile([ERA, OUT], F16)
            nc.sync.dma_start(out=bt_sb[:], in_=btd[:])
            sel_sb = consts.tile([E, ER], F32)
            nc.sync.dma_start(out=sel_sb[:], in_=seld[:])

            w_tiles = {}

            def load_w(ot):
                w_sb = wpool.tile([P, KT, P], F16, tag="w", name="w_sb")
                nc.sync.dma_start(out=w_sb[:], in_=wd[ot])
                w_tiles[ot] = w_sb

            # first two W tiles before the bulk x load: o-tile 0 can start
            # as soon as phase-1 finishes on the PE
            load_w(0)
            load_w(1)

            # Resident activations: x^T tiled (p=i%128, k=i//128, t), fp16, 8 MiB.
            x_sb = consts.tile([P, KT, TPC], F16)
            for k in range(KT):
                nc.sync.dma_start(out=x_sb[:, k, :], in_=xd[:, k, :])

            ones_e1 = consts.tile([E, 1], F32)
            nc.vector.memset(ones_e1[:], 1.0)
            ones_1e = consts.tile([1, E], F32)
            nc.vector.memset(ones_1e[:], 1.0)
            # Gated low-rank projection, fp16, rows 0..31 = wproj, row 32 = ones.
            wp_sb = consts.tile([ERA, TPC], F16)
            nc.vector.memset(wp_sb[ER : ER + 1, :], 1.0)

            # ---------- phase 1: proj + router matmuls ----------
            pps = []
            for t in range(NS_):
                tsl = slice(t * NSLAB, (t + 1) * NSLAB)
                # rows 0..31: proj^T (er, t); rows 32..35: router logits (e, t)
                pp = psum_proj.tile(
                    [ER + E, NSLAB], F32, tag=f"pp{t}", name=f"pp{t}"
                )
                for k in range(KT):
                    nc.tensor.matmul(
                        pp[:],
                        art_sb[:, k, :],
                        x_sb[:, k, tsl],
                        start=(k == 0),
                        stop=(k == KT - 1),
                    )
                pps.append(pp)

            def gating(t):
                # softmax over the 4 expert rows (no max-sub: |logit| < ~8),
                # partition reductions/broadcasts done with tiny PE matmuls
                tsl = slice(t * NSLAB, (t + 1) * NSLAB)
                pp = pps[t]
                e_sb = small.tile([E, NSLAB], F32, tag="e", name="e_sb")
                nc.scalar.activation(
                    e_sb[:], pp[ER : ER + E, :], mybir.ActivationFunctionType.Exp
                )
                s_ps = psum_proj.tile([1, NSLAB], F32, tag="gat", name="s_ps")
                nc.tensor.matmul(s_ps[:], ones_e1[:], e_sb[:])  # sum_e exp
                r_sb = small.tile([1, NSLAB], F32, tag="r", name="r_sb")
                nc.vector.reciprocal(r_sb[:], s_ps[:])
                r4_ps = psum_proj.tile([E, NSLAB], F32, tag="gat", name="r4_ps")
                nc.tensor.matmul(r4_ps[:], ones_1e[:], r_sb[:])  # bcast to 4 rows
                g4_sb = small.tile([E, NSLAB], F32, tag="g4", name="g4_sb")
                nc.vector.tensor_mul(g4_sb[:], e_sb[:], r4_ps[:])
                # (SCALE * gate)[er, t] via 0/1*SCALE selection matmul
                g32_ps = psum_proj.tile([ER, NSLAB], F32, tag="gat", name="g32_ps")
                nc.tensor.matmul(g32_ps[:], sel_sb[:], g4_sb[:])
                # walrus: tensor_tensor may read at most one operand from PSUM
                g32_sb = small.tile([ER, NSLAB], F32, tag="g32s", name="g32_sb")
                nc.vector.tensor_copy(g32_sb[:], g32_ps[:])
                nc.vector.tensor_mul(wp_sb[0:ER, tsl], pp[0:ER, :], g32_sb[:])

            # ---------- phase 2: base matmul + lora + bias ----------
            def base_kloop(ot):
                if ot not in w_tiles:
                    load_w(ot)
                pots = [
                    psum_base.tile([P, NSLAB], F32, tag=f"po{t}", name=f"po{t}")
                    for t in range(NS_)
                ]
                for k in range(KT):
                    for t in range(NS_):
                        nc.tensor.matmul(
                            pots[t][:],
                            w_tiles[ot][:, k, :],
                            x_sb[:, k, t * NSLAB : (t + 1) * NSLAB],
                            start=(k == 0),
                            stop=False,
                        )
                return pots

            def base_tail(ot, pots):
                osl = slice(ot * P, (ot + 1) * P)
                for t in range(NS_):
                    nc.tensor.matmul(
                        pots[t][:],
                        bt_sb[:, osl],
                        wp_sb[:, t * NSLAB : (t + 1) * NSLAB],
                        start=False,
                        stop=True,
                    )
                o_sb = opool.tile([P, TPC], F32, tag="o", name="o_sb")
                for t in range(NS_):
                    nc.vector.tensor_copy(
                        o_sb[:, t * NSLAB : (t + 1) * NSLAB], pots[t][:]
                    )
                nc.sync.dma_start(out=od[osl, :], in_=o_sb[:])
                del w_tiles[ot]

            for rep in range(reps):
                if rep == 0:
                    # o-tile 0's k-loop keeps the PE busy during the gating chain
                    pots0 = base_kloop(0)
                    for t in range(NS_):
                        gating(t)
                    base_tail(0, pots0)
                    start_ot = 1
                else:
                    start_ot = 0
                for ot in range(start_ot, OT):
                    pots = base_kloop(ot)
                    base_tail(ot, pots)

    nc.compile()
    if POST_COMPILE is not None:
        POST_COMPILE(nc)
    else:
        prune_mm_updates(nc)
        dedup_ldweights(nc)
    return nc


def dedup_ldweights(nc):
    """Delete redundant consecutive InstLdweights from the PE stream.

    After bacc lowering every matmul is a standalone InstLdweights followed
    by a non-self-loading InstMatmult.  When two MMs in a row use the same
    stationary tile (the two 512-token slabs of one (o-tile, k) pair), the
    second load is a no-op reload of weights already in the array.  Deleting
    it is safe when (a) its full AP signature matches the immediately
    preceding InstLdweights, (b) only non-self-loading InstMatmults sit
    between them (nothing else touched the array or SBUF ordering), and
    (c) the load carries no semaphore waits/updates.  Any rewrite of the
    underlying SBUF region is ordered after the *later* consumers by the
    tile framework's region tracking, so the weight contents cannot change
    between the two loads.
    """
    from concourse import mybir

    def sig(i):
        ap = i.ins[0]
        return (
            ap.memref,
            ap.offset,
            str(ap.ap),
            str(ap.dtype),
            str(i.tile_position),
            str(i.tile_size),
            str(i.perf_mode),
            str(i.is_transpose),
        )

    removed = 0
    for b in nc.m.functions[0].blocks:
        last = None
        keep = []
        for i in b.instructions:
            if i.engine != mybir.EngineType.PE:
                keep.append(i)
                continue
            if isinstance(i, mybir.InstLdweights):
                si = i.sync_info
                clean = si is None or (not si.on_wait and not si.on_update)
                if clean and last is not None and sig(i) == last:
                    removed += 1
                    continue  # drop the redundant reload
                last = sig(i)
                keep.append(i)
            elif (
                isinstance(i, mybir.InstMatmult)
                and getattr(i, "ldweights", None) is False
            ):
                keep.append(i)
            else:
                last = None
                keep.append(i)
        b.instructions[:] = keep
    return removed


def prune_mm_updates(nc):
    """Remove PE-matmul sem updates that no waiter's threshold references.

    Every tile-emitted matmul carries a `PE_sem++@complete`; the hardware
    retires that sem write on the PE sequencer, and when the next instruction
    is another matmul the write stalls its issue (~40ns measured: bare MM->MM
    gap 91ns vs 53ns with an LDWEIGHTS between).  Waiters (DVE copies, DMA
    recycles) reference only ~100 distinct cumulative counts, so all other
    increments are unobservable.  Keep an update only at referenced
    positions and renumber every wait threshold to the new cumulative count.
    Kept updates remain on the same instructions, so every waiter still
    unblocks at exactly the same matmul completion as before.
    """
    from concourse import mybir

    SEM_CANDIDATES = []
    # find sems updated exclusively by PE InstMatmult via sem-inc
    upd_by = {}
    for f in nc.m.functions:
        for b in f.blocks:
            for i in b.instructions:
                si = i.sync_info
                if not si:
                    continue
                for u in si.on_update:
                    rec = upd_by.setdefault(u.id, {"mm": 0, "other": 0})
                    if (
                        i.engine == mybir.EngineType.PE
                        and isinstance(i, mybir.InstMatmult)
                        and u.update_mode == "sem-inc"
                        and (u.update_value or 1) == 1
                    ):
                        rec["mm"] += 1
                    else:
                        rec["other"] += 1
    for sem_id, rec in upd_by.items():
        if rec["mm"] > 0 and rec["other"] == 0:
            SEM_CANDIDATES.append(sem_id)

    total_removed = 0
    for sem_id in SEM_CANDIDATES:
        # ordered updater positions (PE program order across blocks)
        updaters = []  # instruction refs in order
        waits = []  # (SyncWait refs)
        ok = True
        for f in nc.m.functions:
            for b in f.blocks:
                for i in b.instructions:
                    si = i.sync_info
                    if not si:
                        continue
                    for u in si.on_update:
                        if u.id == sem_id:
                            updaters.append(i)
                    for w in si.on_wait:
                        if w.id == sem_id:
                            if (
                                w.wait_mode != "sem-ge-imm"
                                or w.wait_reg is not None
                                or w.wait_value is None
                            ):
                                ok = False
                            waits.append(w)
        if not ok or not waits:
            continue
        n = len(updaters)
        keep = set()
        for w in waits:
            t = w.wait_value
            if t < 1 or t > n:
                ok = False
                break
            keep.add(t)
        if not ok:
            continue
        keep.add(n)  # preserve the final count for any implicit consumers
        # renumber
        sorted_keep = sorted(keep)
        new_count = {}
        for rank, pos in enumerate(sorted_keep, start=1):
            new_count[pos] = rank
        for w in waits:
            w.wait_value = new_count[w.wait_value]
        for pos, inst in enumerate(updaters, start=1):
            if pos not in keep:
                si = inst.sync_info
                si.on_update = [
                    u for u in si.on_update if u.id != sem_id
                ]
                total_removed += 1
    return total_removed


def consolidate_mm_updates(nc):
    """Move per-matmul semaphore completion updates to accumulation-group ends.

    Every matmul the tile framework emits carries a `sem++@complete` update;
    on hardware each update costs ~26ns of PE sequencer time (see the
    pack-tail model in the tensor-engine guide), which over ~2100 matmuls is
    ~55us of pure overhead.  Matmuls complete in program order, so moving the
    increments of the stop=False matmuls of an accumulation group onto the
    group's final stop=True matmul (with a summed update_value) is
    semantically conservative: every waiter still sees the same final counts,
    just potentially a few instructions later.  Waiters whose thresholds fall
    mid-group (e.g. the W-tile DMA recycling a pool buffer) unblock at the
    group end instead — with bufs=3 prefetch headroom that slack is never on
    the critical path.  Only runs of {InstMatmult, InstLdweights} are
    touched; a segment that does not end in a stop=True matmul is left as-is.
    """
    from concourse import mybir

    moved = 0
    for b in nc.m.functions[0].blocks:
        pending = []  # stripped SyncUpdate objects awaiting a stop=True MM
        stripped = []  # (instruction, saved updates) to restore if no flush
        for i in b.instructions:
            if i.engine != mybir.EngineType.PE:
                continue
            if isinstance(i, mybir.InstLdweights):
                continue
            if isinstance(i, mybir.InstMatmult):
                si = i.sync_info
                if i.stop_tensor_calc:
                    if pending:
                        ups = list(si.on_update) if si is not None else []
                        for u in pending:
                            for tgt in ups:
                                if (
                                    tgt.sync_type == u.sync_type
                                    and tgt.id == u.id
                                    and tgt.update_mode == u.update_mode
                                ):
                                    tgt.update_value = (
                                        tgt.update_value or 1
                                    ) + (u.update_value or 1)
                                    break
                            else:
                                ups.append(u)
                        if si is None:
                            i.sync_info = mybir.SyncInfo(
                                on_wait=[], on_update=ups
                            )
                        else:
                            si.on_update = ups
                        pending = []
                        stripped = []
                else:
                    if si is not None and si.on_update:
                        pending.extend(si.on_update)
                        moved += len(si.on_update)
                        stripped.append((i, list(si.on_update)))
                        si.on_update = []
            else:
                # A non-MM PE instruction (event sem, drain, branch) ends the
                # run; restore any updates not yet flushed so ordering
                # relative to this instruction is preserved.
                for inst, ups in stripped:
                    inst.sync_info.on_update = ups
                    moved -= len(ups)
                pending = []
                stripped = []
        for inst, ups in stripped:
            inst.sync_info.on_update = ups
            moved -= len(ups)
    return moved


def get_nc():
    global _NC
    if _NC is None:
        _NC = build_nc()
    return _NC


def _prep_shared(W, b, A, Bm, Wr):
    # W (OUT, IN) -> wd[ot, p, k, o] = W[ot*128+o, k*128+p], fp16, contiguous
    wd = np.ascontiguousarray(
        W.reshape(OT, P, KT, P).transpose(0, 3, 2, 1).astype(np.float16)
    )
    # [A (E,R,IN) flattened; Wr (E,IN)] -> art[p, k, j] = AR[j, k*128+p]
    ar = np.concatenate([A.reshape(ER, IN), Wr], axis=0)  # (36, IN)
    artd = np.ascontiguousarray(
        ar.T.reshape(KT, P, ER + E).transpose(1, 0, 2).astype(np.float16)
    )
    # Bcat rows er = Bm[e,:,r]; row 32 = bias
    bt = np.concatenate([Bm.transpose(0, 2, 1).reshape(ER, OUT), b[None, :]], axis=0)
    btd = np.ascontiguousarray(bt.astype(np.float16))
    sel = np.zeros((E, ER), np.float32)
    for e in range(E):
        sel[e, e * R : (e + 1) * R] = SCALE
    return wd, artd, btd, sel


def _prep_x_shard(xt, c):
    xs = xt[c * TPC : (c + 1) * TPC]  # (TPC, IN)
    return np.ascontiguousarray(
        xs.T.reshape(KT, P, TPC).transpose(1, 0, 2).astype(np.float16)
    )


def make_in_maps(x, W, b, A, Bm, Wr):
    xt = np.asarray(x, np.float32).reshape(TOK, IN)
    wd, artd, btd, sel = _prep_shared(
        np.asarray(W, np.float32),
        np.asarray(b, np.float32),
        np.asarray(A, np.float32),
        np.asarray(Bm, np.float32),
        np.asarray(Wr, np.float32),
    )
    return [
        {
            "xd": _prep_x_shard(xt, c),
            "wd": wd,
            "artd": artd,
            "btd": btd,
            "seld": sel,
        }
        for c in range(N_CORES)
    ]


def gather_out(results):
    # per-core od is (OUT, TPC); tokens are sharded contiguously
    return np.concatenate([r["od"].T for r in results], axis=0).reshape(B, S, OUT)


def kernel(x, W, b, A, Bm, Wr, _trace=False):
    nc = get_nc()
    in_maps = make_in_maps(x, W, b, A, Bm, Wr)
    res = bass_utils.run_bass_kernel_spmd(
        nc, in_maps, core_ids=list(range(N_CORES)), trace=_trace
    )
    out = gather_out(res.results)
    if _trace:
        return out, res
    return out

